# revision 9
# baseline (speedup 1.0000x reference)
"""GCN (2-layer, PyG GCNConv semantics) on 8 Trainium2 NeuronCores.

Strategy (graph/data parallel, destination-bucketed):
  - Nodes are sorted by total in-degree and dealt round-robin to the 8
    cores (6250 each, padded to 6400 = 50 tiles of 128), so every core's
    tile t holds nodes of near-identical degree and the SPMD-shared
    per-tile chunk counts K[t] carry almost no padding.
  - Normalization is factored per-node: out = Dinv (A+I) Dinv (x@W), so
    the gather tables hold dinv-prescaled features, aggregation is an
    unweighted sum, and the result is dinv-postscaled. For layer 2 the
    table is dinv*relu(out1) = relu(rec*agg1) (b1==0 fast path), so both
    layers gather from a bf16 [51200, 128] table with 256-byte rows.
  - dma_gather indices are int16 (<= 32767 rows), so each destination
    tile gathers from two overlapping windows of the table: rows
    [0, 32768) and [18432, 51200). Edges whose source falls in the
    overlap are assigned to either window per-tile to minimize
    K_lo + K_hi (flex rebalancing).
  - Gathered chunks accumulate into PSUM with bf16 identity matmuls -
    the partition index IS the destination, so scatter is free. Layer 2
    accumulates the TRANSPOSE (lhsT=msg, rhs=ident) so the W2 transform
    is a single matmul per tile with no extra transpose.
  - Gathers round-robin over 4 SWDGE queues with deep buffering: the
    Q7 descriptor generators are the bottleneck (~4.1 ns/desc/queue);
    4 queues + depth-8 pipelining reach ~1.15 ns/desc at 256 B/row.
  - Both layers share one index table (same graph); transformed tables
    are AllGathered (halo exchange) between phases.
"""

import numpy as np
import ml_dtypes

import concourse.bacc as bacc
import concourse.bass as bass
import concourse.mybir as mybir
import concourse.tile as tile
from concourse import bass_utils
from concourse.bass import ts
from concourse.masks import make_identity

N = 50000
F0, F1, F2 = 512, 128, 64
NCORES = 8
NSH = N // NCORES          # 6250 real nodes per core
NP = 6400                  # padded nodes per core (50 tiles of 128)
NT = NP // 128             # 50 dest tiles per core
TBL = NCORES * NP          # 51200 rows in the gathered tables
WLO_END = 32768            # lo window: table rows [0, 32768)
WHI_BASE = TBL - 32768     # hi window: table rows [18432, 51200)
PAD_LO = NP - 1            # core 0's last pad row - all-zero, in lo window
PAD_HI = TBL - 1 - WHI_BASE  # core 7's last pad row, relative to hi base
NQ = 4                     # SWDGE queues
MBUFS = 6                  # gather pipeline depth per half
F32 = mybir.dt.float32
BF16 = mybir.dt.bfloat16
I16 = mybir.dt.int16
BF = ml_dtypes.bfloat16

_TRACE = False
_LAST = None               # BassKernelResults of the most recent run


def _wrap16(flat_idx):
    """dma_gather index layout: element i at [i%16, i//16], replicated to
    128 partitions (one copy per GpSimd core)."""
    n = len(flat_idx)
    a = np.zeros((16, n // 16), np.int16)
    i = np.arange(n)
    a[i % 16, i // 16] = flat_idx.astype(np.int16)
    return np.tile(a, (8, 1))


def _host_prep(x, edge_index, W1, b1, W2, b2):
    src = np.asarray(edge_index[0], dtype=np.int64)
    dst = np.asarray(edge_index[1], dtype=np.int64)
    x = np.asarray(x, dtype=np.float32)

    deg = np.bincount(dst, minlength=N) + 1  # self-loops included

    # Sort by total degree; deal round-robin so all cores share one K[t].
    gorder = np.argsort(-deg, kind="stable")
    rank = np.empty(N, np.int64)
    rank[gorder] = np.arange(N)
    perm = np.empty(N, dtype=np.int64)       # perm[c*NSH + p] = node id
    perm[(rank % NCORES) * NSH + rank // NCORES] = np.arange(N)
    row = NP * (rank % NCORES) + rank // NCORES  # padded table row per node

    # All edges incl self-loops, in padded coords.
    loop = np.arange(N, dtype=np.int64)
    all_src = np.concatenate([src, loop])
    all_dst = np.concatenate([dst, loop])
    srow = row[all_src]
    drow = row[all_dst]
    # window class: 0 = lo-only, 1 = flex (either), 2 = hi-only
    cls = np.where(srow < WHI_BASE, 0, np.where(srow < WLO_END, 1, 2))

    # per-node class counts, laid out by rank band
    NRANK = NCORES * NP
    Ab = np.zeros(NRANK, np.int64)
    Bb = np.zeros(NRANK, np.int64)
    Cb = np.zeros(NRANK, np.int64)
    Ab[rank] = np.bincount(all_dst[cls == 0], minlength=N)
    Bb[rank] = np.bincount(all_dst[cls == 1], minlength=N)
    Cb[rank] = np.bincount(all_dst[cls == 2], minlength=N)

    # per-tile rebalance of flex edges: klo_p = a_p + x_p <= KLO,
    # khi_p = c_p + b_p - x_p <= KHI, x_p = clip(KLO - a_p, 0, b_p)
    KLO = np.zeros(NT, np.int64)
    KHI = np.zeros(NT, np.int64)
    for t in range(NT):
        a = Ab[t * 1024:(t + 1) * 1024]
        bb = Bb[t * 1024:(t + 1) * 1024]
        c = Cb[t * 1024:(t + 1) * 1024]
        best = 1 << 30
        bl = bh = 0
        for L in range(int(a.max()), int((a + bb).max()) + 1):
            xf = np.minimum(bb, L - a)
            kh = int((c + bb - xf).max())
            if L + kh < best:
                best, bl, bh = L + kh, L, kh
        KLO[t], KHI[t] = bl, bh

    # Per-edge half assignment: sort edges by (dst, cls); for each dst the
    # first a + x edges go LO, the rest HI.
    order = np.lexsort((cls, all_dst))
    sd = all_dst[order]
    ss = srow[order]
    e_rank = rank[sd]                       # dst rank per sorted edge
    t_of = e_rank // 1024                   # dst tile
    a_of = Ab[e_rank]
    b_of = Bb[e_rank]
    x_of = np.clip(KLO[t_of] - a_of, 0, b_of)
    starts = np.searchsorted(sd, np.arange(N))
    j = np.arange(len(sd)) - starts[sd]     # edge index within its dst
    to_lo = j < (a_of + x_of)
    # slot number within the half
    jlo = j
    jhi = j - (a_of + x_of)

    # destination coordinates
    c_of = e_rank % NCORES
    p_of = (e_rank // NCORES) % 128

    offs_lo = np.concatenate([[0], np.cumsum(KLO)]).astype(np.int64)
    offs_hi = np.concatenate([[0], np.cumsum(KHI)]).astype(np.int64)
    idx_lo = np.full((NCORES, 128, int(offs_lo[-1])), PAD_LO, dtype=np.int64)
    idx_hi = np.full((NCORES, 128, int(offs_hi[-1])), PAD_HI, dtype=np.int64)
    m = to_lo
    idx_lo[c_of[m], p_of[m], offs_lo[t_of[m]] + jlo[m]] = ss[m]
    m = ~to_lo
    idx_hi[c_of[m], p_of[m], offs_hi[t_of[m]] + jhi[m]] = ss[m] - WHI_BASE

    def wrap_core(idx_c, K, offs):
        blocks = []
        for t in range(NT):
            if K[t] == 0:
                continue
            blk = idx_c[:, offs[t]:offs[t + 1]]       # [128, K[t]]
            flat = blk.T.reshape(-1)                  # order (j, p)
            blocks.append(_wrap16(flat))
        return np.ascontiguousarray(np.concatenate(blocks, axis=1))

    w1b = np.asarray(W1, np.float32).astype(BF).reshape(4, 128, F1)
    w2b = np.asarray(W2, np.float32).astype(BF)
    in_maps = []
    for c in range(NCORES):
        pc = perm[c * NSH:(c + 1) * NSH]
        xp = np.zeros((NP, F0), dtype=np.float32)
        xp[:NSH] = x[pc]
        xt4 = np.ascontiguousarray(xp.T.reshape(4, 128, NP).astype(BF))
        degp = np.ones(NP, dtype=np.float32)
        degp[:NSH] = deg[pc].astype(np.float32)
        degT = np.ascontiguousarray(degp.reshape(NT, 128).T)  # [128, NT]
        m = {
            "xt4": xt4,
            "w1r": np.ascontiguousarray(w1b),
            "w2": np.ascontiguousarray(w2b),
            "degT": degT,
            "idxlo": wrap_core(idx_lo[c], KLO, offs_lo),
            "idxhi": wrap_core(idx_hi[c], KHI, offs_hi),
        }
        in_maps.append(m)

    return in_maps, perm, KLO, KHI


def _build(KLO, KHI, wlo, whi, b1_zero, b2_zero):
    Relu = mybir.ActivationFunctionType.Relu
    Copy = mybir.ActivationFunctionType.Copy
    Sqrt = mybir.ActivationFunctionType.Sqrt

    nc = bacc.Bacc("TRN2", target_bir_lowering=False, num_devices=NCORES,
                   num_swdge_queues=NQ)

    xt4_d = nc.dram_tensor("xt4", [4, 128, NP], BF16, kind="ExternalInput")
    w1r_d = nc.dram_tensor("w1r", [4, 128, F1], BF16, kind="ExternalInput")
    w2_d = nc.dram_tensor("w2", [F1, F2], BF16, kind="ExternalInput")
    degT_d = nc.dram_tensor("degT", [128, NT], F32, kind="ExternalInput")
    ilo_d = nc.dram_tensor("idxlo", [128, wlo], I16, kind="ExternalInput")
    ihi_d = nc.dram_tensor("idxhi", [128, whi], I16, kind="ExternalInput")
    b1r_d = b2r_d = None
    if not b1_zero:
        b1r_d = nc.dram_tensor("b1r", [128, F1], F32, kind="ExternalInput")
    if not b2_zero:
        b2r_d = nc.dram_tensor("b2r", [128, F2], F32, kind="ExternalInput")
    out_d = nc.dram_tensor("out", [NP, F2], F32, kind="ExternalOutput")

    h1_loc = nc.dram_tensor("h1_loc", [NP, F1], BF16, kind="Internal")
    h1_full = nc.dram_tensor("h1_full", [TBL, F1], BF16, kind="Internal",
                             addr_space="Shared")
    t2_loc = nc.dram_tensor("t2_loc", [NP, F1], BF16, kind="Internal")
    t2_full = nc.dram_tensor("t2_full", [TBL, F1], BF16, kind="Internal",
                             addr_space="Shared")

    rg = [list(range(NCORES))]
    KLMAX, KHMAX = int(max(KLO)), int(max(KHI))
    qctr = [0]

    with tile.TileContext(nc, num_cores=NCORES) as tc:
        with (
            tc.tile_pool(name="const", bufs=1) as cpool,
            tc.tile_pool(name="stream", bufs=3) as spool,
            tc.tile_pool(name="msg", bufs=MBUFS) as mpool,
            tc.tile_pool(name="psA", bufs=3, space="PSUM") as psA,
            tc.tile_pool(name="psW", bufs=2, space="PSUM") as psW,
        ):
            # ---- constants -------------------------------------------------
            w1sb = cpool.tile([128, 4, F1], BF16)
            nc.sync.dma_start(out=w1sb[:], in_=w1r_d[:].rearrange("k p f -> p k f"))
            w2sb = cpool.tile([128, F2], BF16)
            nc.sync.dma_start(out=w2sb[:], in_=w2_d[:])
            degsb = cpool.tile([128, NT], F32)
            nc.sync.dma_start(out=degsb[:], in_=degT_d[:])
            ilosb = cpool.tile([128, wlo], I16)
            nc.sync.dma_start(out=ilosb[:], in_=ilo_d[:])
            ihisb = cpool.tile([128, whi], I16)
            nc.sync.dma_start(out=ihisb[:], in_=ihi_d[:])
            ident = cpool.tile([128, 128], BF16)
            make_identity(nc, ident[:])
            b1sb = b2sb = None
            if not b1_zero:
                b1sb = cpool.tile([128, F1], F32)
                nc.sync.dma_start(out=b1sb[:], in_=b1r_d[:])
            if not b2_zero:
                b2sb = cpool.tile([128, F2], F32)
                nc.sync.dma_start(out=b2sb[:], in_=b2r_d[:])

            rec = cpool.tile([128, NT], F32)
            nc.vector.reciprocal(rec[:], degsb[:])
            dinv = cpool.tile([128, NT], F32)
            nc.scalar.activation(dinv[:], rec[:], Sqrt)
            zero1 = cpool.tile([128, F1], BF16)
            nc.gpsimd.memset(zero1[:], 0.0)

            # ---- phase B: h1 = (x @ W1) * dinv ------------------------------
            for t in range(NT):
                xt = spool.tile([128, 4, 128], BF16, tag="xt")
                nc.sync.dma_start(
                    out=xt[:],
                    in_=xt4_d[:, :, ts(t, 128)].rearrange("k p n -> p k n"))
                ph = psA.tile([128, F1], F32, tag="po")
                for k in range(4):
                    nc.tensor.matmul(ph[:], lhsT=xt[:, k, :], rhs=w1sb[:, k, :],
                                     start=(k == 0), stop=(k == 3))
                h1t = spool.tile([128, F1], BF16, tag="h1t")
                nc.scalar.activation(h1t[:], ph[:], Copy, scale=dinv[:, t:t + 1])
                nc.sync.dma_start(out=h1_loc[ts(t, 128), :], in_=h1t[:])

            # ---- AllGather h1 ----------------------------------------------
            nc.gpsimd.collective_compute(
                "AllGather", mybir.AluOpType.bypass, replica_groups=rg,
                ins=[h1_loc[:]], outs=[h1_full[:]])

            # ---- gather + accumulate helper --------------------------------
            def aggregate(table, transposed, consume):
                olo = ohi = 0
                for t in range(NT):
                    klo, khi = int(KLO[t]), int(KHI[t])
                    if klo + khi == 0:
                        consume(t, None)
                        continue
                    mlo = mhi = None
                    if klo:
                        nlo = 128 * klo
                        mlo = mpool.tile([128, KLMAX, F1], BF16, tag="mlo")
                        nc.gpsimd.dma_gather(
                            out_ap=mlo[:, :klo, :], in_ap=table[:WLO_END, :],
                            idxs_ap=ilosb[:, olo:olo + nlo // 16],
                            num_idxs=nlo, num_idxs_reg=nlo, elem_size=F1,
                            single_packet=False, queue_num=qctr[0] % NQ)
                        qctr[0] += 1
                        olo += nlo // 16
                    if khi:
                        nhi = 128 * khi
                        mhi = mpool.tile([128, KHMAX, F1], BF16, tag="mhi")
                        nc.gpsimd.dma_gather(
                            out_ap=mhi[:, :khi, :], in_ap=table[WHI_BASE:, :],
                            idxs_ap=ihisb[:, ohi:ohi + nhi // 16],
                            num_idxs=nhi, num_idxs_reg=nhi, elem_size=F1,
                            single_packet=False, queue_num=qctr[0] % NQ)
                        qctr[0] += 1
                        ohi += nhi // 16
                    po = psA.tile([128, F1], F32, tag="po")
                    nk = klo + khi
                    for j in range(nk):
                        rhs = mlo[:, j, :] if j < klo else mhi[:, j - klo, :]
                        if transposed:
                            nc.tensor.matmul(po[:], lhsT=rhs, rhs=ident[:],
                                             start=(j == 0), stop=(j == nk - 1))
                        else:
                            nc.tensor.matmul(po[:], lhsT=ident[:], rhs=rhs,
                                             start=(j == 0), stop=(j == nk - 1))
                    consume(t, po)

            # ---- layer 1 aggregate: t2 = relu(rec * agg) -------------------
            def consume1(t, po):
                if po is None:  # all-pad tile: table rows must be zero
                    nc.sync.dma_start(out=t2_loc[ts(t, 128), :], in_=zero1[:])
                    return
                t2t = spool.tile([128, F1], BF16, tag="t2t")
                if b1_zero:
                    nc.scalar.activation(t2t[:], po[:], Relu,
                                         scale=rec[:, t:t + 1])
                else:
                    tmp = spool.tile([128, F1], F32, tag="tmp1")
                    nc.scalar.activation(tmp[:], po[:], Copy,
                                         scale=dinv[:, t:t + 1])
                    nc.vector.tensor_tensor(out=tmp[:], in0=tmp[:], in1=b1sb[:],
                                            op=mybir.AluOpType.add)
                    tmp2 = spool.tile([128, F1], F32, tag="tmp2")
                    nc.scalar.activation(tmp2[:], tmp[:], Relu)
                    nc.scalar.activation(t2t[:], tmp2[:], Copy,
                                         scale=dinv[:, t:t + 1])
                nc.sync.dma_start(out=t2_loc[ts(t, 128), :], in_=t2t[:])

            aggregate(h1_full, False, consume1)

            # ---- AllGather t2 ----------------------------------------------
            nc.gpsimd.collective_compute(
                "AllGather", mybir.AluOpType.bypass, replica_groups=rg,
                ins=[t2_loc[:]], outs=[t2_full[:]])

            # ---- layer 2 aggregate (transposed) + W2 -----------------------
            def consume2(t, poT):
                if poT is None:  # all-pad tile: host never reads these rows
                    return
                pT = spool.tile([128, F1], BF16, tag="pT")
                nc.scalar.copy(pT[:], poT[:])
                raw = psW.tile([128, F2], F32, tag="raw")
                nc.tensor.matmul(raw[:], lhsT=pT[:], rhs=w2sb[:],
                                 start=True, stop=True)
                o2t = spool.tile([128, F2], F32, tag="o2t")
                nc.scalar.activation(o2t[:], raw[:], Copy,
                                     scale=dinv[:, t:t + 1])
                if not b2_zero:
                    nc.vector.tensor_tensor(out=o2t[:], in0=o2t[:], in1=b2sb[:],
                                            op=mybir.AluOpType.add)
                nc.sync.dma_start(out=out_d[ts(t, 128), :], in_=o2t[:])

            aggregate(t2_full, True, consume2)

    nc.compile()
    return nc


def kernel(x, edge_index, W1, b1, W2, b2):
    global _LAST
    b1 = np.asarray(b1, np.float32)
    b2 = np.asarray(b2, np.float32)
    in_maps, perm, KLO, KHI = _host_prep(x, edge_index, W1, b1, W2, b2)

    b1_zero = bool(np.all(b1 == 0))
    b2_zero = bool(np.all(b2 == 0))
    if not b1_zero:
        for m in in_maps:
            m["b1r"] = np.ascontiguousarray(np.tile(b1[None, :], (128, 1)))
    if not b2_zero:
        for m in in_maps:
            m["b2r"] = np.ascontiguousarray(np.tile(b2[None, :], (128, 1)))

    wlo = in_maps[0]["idxlo"].shape[1]
    whi = in_maps[0]["idxhi"].shape[1]
    nc = _build(KLO, KHI, wlo, whi, b1_zero, b2_zero)

    res = bass_utils.run_bass_kernel_spmd(
        nc, in_maps, core_ids=list(range(NCORES)), trace=_TRACE)
    _LAST = res

    out = np.empty((N, F2), dtype=np.float32)
    for c in range(NCORES):
        pc = perm[c * NSH:(c + 1) * NSH]
        out[pc] = res.results[c]["out"][:NSH]
    return out


# revision 10
# speedup vs baseline: 1.0279x; 1.0279x over previous
"""GCN (2-layer, PyG GCNConv semantics) on 8 Trainium2 NeuronCores.

Strategy (graph/data parallel, destination-bucketed):
  - Nodes are sorted by total in-degree and dealt round-robin to the 8
    cores (6250 each, padded to 6400 = 50 tiles of 128), so every core's
    tile t holds nodes of near-identical degree and the SPMD-shared
    per-tile chunk counts K[t] carry almost no padding.
  - Normalization is factored per-node: out = Dinv (A+I) Dinv (x@W), so
    the gather tables hold dinv-prescaled features, aggregation is an
    unweighted sum, and the result is dinv-postscaled. For layer 2 the
    table is dinv*relu(out1) = relu(rec*agg1) (b1==0 fast path), so both
    layers gather from a bf16 [51200, 128] table with 256-byte rows.
  - dma_gather indices are int16 (<= 32767 rows), so each destination
    tile gathers from two overlapping windows of the table: rows
    [0, 32768) and [18432, 51200). Edges whose source falls in the
    overlap are assigned to either window per-tile to minimize
    K_lo + K_hi (flex rebalancing).
  - Gathered chunks accumulate into PSUM with bf16 identity matmuls -
    the partition index IS the destination, so scatter is free. Layer 2
    accumulates the TRANSPOSE (lhsT=msg, rhs=ident) so the W2 transform
    is a single matmul per tile with no extra transpose.
  - Gathers round-robin over 4 SWDGE queues with deep buffering: the
    Q7 descriptor generators are the bottleneck (~4.1 ns/desc/queue);
    4 queues + depth-8 pipelining reach ~1.15 ns/desc at 256 B/row.
  - Both layers share one index table (same graph); transformed tables
    are AllGathered (halo exchange) between phases.
"""

import numpy as np
import ml_dtypes

import concourse.bacc as bacc
import concourse.bass as bass
import concourse.mybir as mybir
import concourse.tile as tile
from concourse import bass_utils
from concourse.bass import ts
from concourse.masks import make_identity

N = 50000
F0, F1, F2 = 512, 128, 64
NCORES = 8
NSH = N // NCORES          # 6250 real nodes per core
NP = 6400                  # padded nodes per core (50 tiles of 128)
NT = NP // 128             # 50 dest tiles per core
TBL = NCORES * NP          # 51200 rows in the gathered tables
WLO_END = 32768            # lo window: table rows [0, 32768)
WHI_BASE = TBL - 32768     # hi window: table rows [18432, 51200)
PAD_LO = NP - 1            # core 0's last pad row - all-zero, in lo window
PAD_HI = TBL - 1 - WHI_BASE  # core 7's last pad row, relative to hi base
NQ = 4                     # SWDGE queues
MBUFS = 8                  # gather pipeline depth per half
F32 = mybir.dt.float32
BF16 = mybir.dt.bfloat16
I16 = mybir.dt.int16
BF = ml_dtypes.bfloat16

_TRACE = False
_LAST = None               # BassKernelResults of the most recent run


def _wrap16(flat_idx):
    """dma_gather index layout: element i at [i%16, i//16], replicated to
    128 partitions (one copy per GpSimd core)."""
    n = len(flat_idx)
    a = np.zeros((16, n // 16), np.int16)
    i = np.arange(n)
    a[i % 16, i // 16] = flat_idx.astype(np.int16)
    return np.tile(a, (8, 1))


def _host_prep(x, edge_index, W1, b1, W2, b2):
    src = np.asarray(edge_index[0], dtype=np.int64)
    dst = np.asarray(edge_index[1], dtype=np.int64)
    x = np.asarray(x, dtype=np.float32)

    deg = np.bincount(dst, minlength=N) + 1  # self-loops included

    # Sort by total degree; deal round-robin so all cores share one K[t].
    gorder = np.argsort(-deg, kind="stable")
    rank = np.empty(N, np.int64)
    rank[gorder] = np.arange(N)
    perm = np.empty(N, dtype=np.int64)       # perm[c*NSH + p] = node id
    perm[(rank % NCORES) * NSH + rank // NCORES] = np.arange(N)
    row = NP * (rank % NCORES) + rank // NCORES  # padded table row per node

    # All edges incl self-loops, in padded coords.
    loop = np.arange(N, dtype=np.int64)
    all_src = np.concatenate([src, loop])
    all_dst = np.concatenate([dst, loop])
    srow = row[all_src]
    drow = row[all_dst]
    # window class: 0 = lo-only, 1 = flex (either), 2 = hi-only
    cls = np.where(srow < WHI_BASE, 0, np.where(srow < WLO_END, 1, 2))

    # per-node class counts, laid out by rank band
    NRANK = NCORES * NP
    Ab = np.zeros(NRANK, np.int64)
    Bb = np.zeros(NRANK, np.int64)
    Cb = np.zeros(NRANK, np.int64)
    Ab[rank] = np.bincount(all_dst[cls == 0], minlength=N)
    Bb[rank] = np.bincount(all_dst[cls == 1], minlength=N)
    Cb[rank] = np.bincount(all_dst[cls == 2], minlength=N)

    # per-tile rebalance of flex edges: klo_p = a_p + x_p <= KLO,
    # khi_p = c_p + b_p - x_p <= KHI, x_p = clip(KLO - a_p, 0, b_p)
    KLO = np.zeros(NT, np.int64)
    KHI = np.zeros(NT, np.int64)
    for t in range(NT):
        a = Ab[t * 1024:(t + 1) * 1024]
        bb = Bb[t * 1024:(t + 1) * 1024]
        c = Cb[t * 1024:(t + 1) * 1024]
        best = 1 << 30
        bl = bh = 0
        for L in range(int(a.max()), int((a + bb).max()) + 1):
            xf = np.minimum(bb, L - a)
            kh = int((c + bb - xf).max())
            if L + kh < best:
                best, bl, bh = L + kh, L, kh
        KLO[t], KHI[t] = bl, bh

    # Per-edge half assignment: sort edges by (dst, cls); for each dst the
    # first a + x edges go LO, the rest HI.
    order = np.lexsort((cls, all_dst))
    sd = all_dst[order]
    ss = srow[order]
    e_rank = rank[sd]                       # dst rank per sorted edge
    t_of = e_rank // 1024                   # dst tile
    a_of = Ab[e_rank]
    b_of = Bb[e_rank]
    x_of = np.clip(KLO[t_of] - a_of, 0, b_of)
    starts = np.searchsorted(sd, np.arange(N))
    j = np.arange(len(sd)) - starts[sd]     # edge index within its dst
    to_lo = j < (a_of + x_of)
    # slot number within the half
    jlo = j
    jhi = j - (a_of + x_of)

    # destination coordinates
    c_of = e_rank % NCORES
    p_of = (e_rank // NCORES) % 128

    offs_lo = np.concatenate([[0], np.cumsum(KLO)]).astype(np.int64)
    offs_hi = np.concatenate([[0], np.cumsum(KHI)]).astype(np.int64)
    idx_lo = np.full((NCORES, 128, int(offs_lo[-1])), PAD_LO, dtype=np.int64)
    idx_hi = np.full((NCORES, 128, int(offs_hi[-1])), PAD_HI, dtype=np.int64)
    m = to_lo
    idx_lo[c_of[m], p_of[m], offs_lo[t_of[m]] + jlo[m]] = ss[m]
    m = ~to_lo
    idx_hi[c_of[m], p_of[m], offs_hi[t_of[m]] + jhi[m]] = ss[m] - WHI_BASE

    def wrap_core(idx_c, K, offs):
        blocks = []
        for t in range(NT):
            if K[t] == 0:
                continue
            blk = idx_c[:, offs[t]:offs[t + 1]]       # [128, K[t]]
            flat = blk.T.reshape(-1)                  # order (j, p)
            blocks.append(_wrap16(flat))
        return np.ascontiguousarray(np.concatenate(blocks, axis=1))

    w1b = np.asarray(W1, np.float32).astype(BF).reshape(4, 128, F1)
    w2b = np.asarray(W2, np.float32).astype(BF)
    in_maps = []
    for c in range(NCORES):
        pc = perm[c * NSH:(c + 1) * NSH]
        xp = np.zeros((NP, F0), dtype=np.float32)
        xp[:NSH] = x[pc]
        xt4 = np.ascontiguousarray(xp.T.reshape(4, 128, NP).astype(BF))
        degp = np.ones(NP, dtype=np.float32)
        degp[:NSH] = deg[pc].astype(np.float32)
        degT = np.ascontiguousarray(degp.reshape(NT, 128).T)  # [128, NT]
        m = {
            "xt4": xt4,
            "w1r": np.ascontiguousarray(w1b),
            "w2": np.ascontiguousarray(w2b),
            "degT": degT,
            "idxlo": wrap_core(idx_lo[c], KLO, offs_lo),
            "idxhi": wrap_core(idx_hi[c], KHI, offs_hi),
        }
        in_maps.append(m)

    return in_maps, perm, KLO, KHI


def _build(KLO, KHI, wlo, whi, b1_zero, b2_zero):
    Relu = mybir.ActivationFunctionType.Relu
    Copy = mybir.ActivationFunctionType.Copy
    Sqrt = mybir.ActivationFunctionType.Sqrt

    nc = bacc.Bacc("TRN2", target_bir_lowering=False, num_devices=NCORES,
                   num_swdge_queues=NQ, dynamic_dma_scratch_size=32768)

    xt4_d = nc.dram_tensor("xt4", [4, 128, NP], BF16, kind="ExternalInput")
    w1r_d = nc.dram_tensor("w1r", [4, 128, F1], BF16, kind="ExternalInput")
    w2_d = nc.dram_tensor("w2", [F1, F2], BF16, kind="ExternalInput")
    degT_d = nc.dram_tensor("degT", [128, NT], F32, kind="ExternalInput")
    ilo_d = nc.dram_tensor("idxlo", [128, wlo], I16, kind="ExternalInput")
    ihi_d = nc.dram_tensor("idxhi", [128, whi], I16, kind="ExternalInput")
    b1r_d = b2r_d = None
    if not b1_zero:
        b1r_d = nc.dram_tensor("b1r", [128, F1], F32, kind="ExternalInput")
    if not b2_zero:
        b2r_d = nc.dram_tensor("b2r", [128, F2], F32, kind="ExternalInput")
    out_d = nc.dram_tensor("out", [NP, F2], F32, kind="ExternalOutput")

    h1_loc = nc.dram_tensor("h1_loc", [NP, F1], BF16, kind="Internal")
    h1_full = nc.dram_tensor("h1_full", [TBL, F1], BF16, kind="Internal",
                             addr_space="Shared")
    t2_loc = nc.dram_tensor("t2_loc", [NP, F1], BF16, kind="Internal")
    t2_full = nc.dram_tensor("t2_full", [TBL, F1], BF16, kind="Internal",
                             addr_space="Shared")

    rg = [list(range(NCORES))]
    KLMAX, KHMAX = int(max(KLO)), int(max(KHI))
    qctr = [0]

    with tile.TileContext(nc, num_cores=NCORES) as tc:
        with (
            tc.tile_pool(name="const", bufs=1) as cpool,
            tc.tile_pool(name="stream", bufs=3) as spool,
            tc.tile_pool(name="msg", bufs=MBUFS) as mpool,
            tc.tile_pool(name="psA", bufs=4, space="PSUM") as psA,
            tc.tile_pool(name="psW", bufs=2, space="PSUM") as psW,
        ):
            # ---- constants -------------------------------------------------
            w1sb = cpool.tile([128, 4, F1], BF16)
            nc.sync.dma_start(out=w1sb[:], in_=w1r_d[:].rearrange("k p f -> p k f"))
            w2sb = cpool.tile([128, F2], BF16)
            nc.sync.dma_start(out=w2sb[:], in_=w2_d[:])
            degsb = cpool.tile([128, NT], F32)
            nc.sync.dma_start(out=degsb[:], in_=degT_d[:])
            ilosb = cpool.tile([128, wlo], I16)
            nc.sync.dma_start(out=ilosb[:], in_=ilo_d[:])
            ihisb = cpool.tile([128, whi], I16)
            nc.sync.dma_start(out=ihisb[:], in_=ihi_d[:])
            ident = cpool.tile([128, 128], BF16)
            make_identity(nc, ident[:])
            b1sb = b2sb = None
            if not b1_zero:
                b1sb = cpool.tile([128, F1], F32)
                nc.sync.dma_start(out=b1sb[:], in_=b1r_d[:])
            if not b2_zero:
                b2sb = cpool.tile([128, F2], F32)
                nc.sync.dma_start(out=b2sb[:], in_=b2r_d[:])

            rec = cpool.tile([128, NT], F32)
            nc.vector.reciprocal(rec[:], degsb[:])
            dinv = cpool.tile([128, NT], F32)
            nc.scalar.activation(dinv[:], rec[:], Sqrt)
            zero1 = cpool.tile([128, F1], BF16)
            nc.gpsimd.memset(zero1[:], 0.0)

            # ---- phase B: h1 = (x @ W1) * dinv ------------------------------
            for t in range(NT):
                xt = spool.tile([128, 4, 128], BF16, tag="xt")
                nc.sync.dma_start(
                    out=xt[:],
                    in_=xt4_d[:, :, ts(t, 128)].rearrange("k p n -> p k n"))
                ph = psA.tile([128, F1], F32, tag="po")
                for k in range(4):
                    nc.tensor.matmul(ph[:], lhsT=xt[:, k, :], rhs=w1sb[:, k, :],
                                     start=(k == 0), stop=(k == 3))
                h1t = spool.tile([128, F1], BF16, tag="h1t")
                nc.scalar.activation(h1t[:], ph[:], Copy, scale=dinv[:, t:t + 1])
                nc.sync.dma_start(out=h1_loc[ts(t, 128), :], in_=h1t[:])

            # ---- AllGather h1 ----------------------------------------------
            nc.gpsimd.collective_compute(
                "AllGather", mybir.AluOpType.bypass, replica_groups=rg,
                ins=[h1_loc[:]], outs=[h1_full[:]])

            # ---- gather + accumulate helper --------------------------------
            def aggregate(table, transposed, consume):
                olo = ohi = 0
                for t in range(NT):
                    klo, khi = int(KLO[t]), int(KHI[t])
                    if klo + khi == 0:
                        consume(t, None)
                        continue
                    mlo = mhi = None
                    if klo:
                        nlo = 128 * klo
                        mlo = mpool.tile([128, KLMAX, F1], BF16, tag="mlo")
                        nc.gpsimd.dma_gather(
                            out_ap=mlo[:, :klo, :], in_ap=table[:WLO_END, :],
                            idxs_ap=ilosb[:, olo:olo + nlo // 16],
                            num_idxs=nlo, num_idxs_reg=nlo, elem_size=F1,
                            single_packet=False, queue_num=qctr[0] % NQ)
                        qctr[0] += 1
                        olo += nlo // 16
                    if khi:
                        nhi = 128 * khi
                        mhi = mpool.tile([128, KHMAX, F1], BF16, tag="mhi")
                        nc.gpsimd.dma_gather(
                            out_ap=mhi[:, :khi, :], in_ap=table[WHI_BASE:, :],
                            idxs_ap=ihisb[:, ohi:ohi + nhi // 16],
                            num_idxs=nhi, num_idxs_reg=nhi, elem_size=F1,
                            single_packet=False, queue_num=qctr[0] % NQ)
                        qctr[0] += 1
                        ohi += nhi // 16
                    po = psA.tile([128, F1], F32, tag="po")
                    nk = klo + khi
                    for j in range(nk):
                        rhs = mlo[:, j, :] if j < klo else mhi[:, j - klo, :]
                        if transposed:
                            nc.tensor.matmul(po[:], lhsT=rhs, rhs=ident[:],
                                             start=(j == 0), stop=(j == nk - 1))
                        else:
                            nc.tensor.matmul(po[:], lhsT=ident[:], rhs=rhs,
                                             start=(j == 0), stop=(j == nk - 1))
                    consume(t, po)

            # ---- layer 1 aggregate: t2 = relu(rec * agg) -------------------
            def consume1(t, po):
                if po is None:  # all-pad tile: table rows must be zero
                    nc.sync.dma_start(out=t2_loc[ts(t, 128), :], in_=zero1[:])
                    return
                t2t = spool.tile([128, F1], BF16, tag="t2t")
                if b1_zero:
                    nc.scalar.activation(t2t[:], po[:], Relu,
                                         scale=rec[:, t:t + 1])
                else:
                    tmp = spool.tile([128, F1], F32, tag="tmp1")
                    nc.scalar.activation(tmp[:], po[:], Copy,
                                         scale=dinv[:, t:t + 1])
                    nc.vector.tensor_tensor(out=tmp[:], in0=tmp[:], in1=b1sb[:],
                                            op=mybir.AluOpType.add)
                    tmp2 = spool.tile([128, F1], F32, tag="tmp2")
                    nc.scalar.activation(tmp2[:], tmp[:], Relu)
                    nc.scalar.activation(t2t[:], tmp2[:], Copy,
                                         scale=dinv[:, t:t + 1])
                nc.sync.dma_start(out=t2_loc[ts(t, 128), :], in_=t2t[:])

            aggregate(h1_full, False, consume1)

            # ---- AllGather t2 ----------------------------------------------
            nc.gpsimd.collective_compute(
                "AllGather", mybir.AluOpType.bypass, replica_groups=rg,
                ins=[t2_loc[:]], outs=[t2_full[:]])

            # ---- layer 2 aggregate (transposed) + W2 -----------------------
            def consume2(t, poT):
                if poT is None:  # all-pad tile: host never reads these rows
                    return
                pT = spool.tile([128, F1], BF16, tag="pT")
                nc.scalar.copy(pT[:], poT[:])
                raw = psW.tile([128, F2], F32, tag="raw")
                nc.tensor.matmul(raw[:], lhsT=pT[:], rhs=w2sb[:],
                                 start=True, stop=True)
                o2t = spool.tile([128, F2], F32, tag="o2t")
                nc.scalar.activation(o2t[:], raw[:], Copy,
                                     scale=dinv[:, t:t + 1])
                if not b2_zero:
                    nc.vector.tensor_tensor(out=o2t[:], in0=o2t[:], in1=b2sb[:],
                                            op=mybir.AluOpType.add)
                nc.sync.dma_start(out=out_d[ts(t, 128), :], in_=o2t[:])

            aggregate(t2_full, True, consume2)

    nc.compile()
    return nc


def kernel(x, edge_index, W1, b1, W2, b2):
    global _LAST
    b1 = np.asarray(b1, np.float32)
    b2 = np.asarray(b2, np.float32)
    in_maps, perm, KLO, KHI = _host_prep(x, edge_index, W1, b1, W2, b2)

    b1_zero = bool(np.all(b1 == 0))
    b2_zero = bool(np.all(b2 == 0))
    if not b1_zero:
        for m in in_maps:
            m["b1r"] = np.ascontiguousarray(np.tile(b1[None, :], (128, 1)))
    if not b2_zero:
        for m in in_maps:
            m["b2r"] = np.ascontiguousarray(np.tile(b2[None, :], (128, 1)))

    wlo = in_maps[0]["idxlo"].shape[1]
    whi = in_maps[0]["idxhi"].shape[1]
    nc = _build(KLO, KHI, wlo, whi, b1_zero, b2_zero)

    res = bass_utils.run_bass_kernel_spmd(
        nc, in_maps, core_ids=list(range(NCORES)), trace=_TRACE)
    _LAST = res

    out = np.empty((N, F2), dtype=np.float32)
    for c in range(NCORES):
        pc = perm[c * NSH:(c + 1) * NSH]
        out[pc] = res.results[c]["out"][:NSH]
    return out
